# revision 46
# baseline (speedup 1.0000x reference)
"""Trainium2 Bass kernel for the MetricLearning pairwise loss.

Reference math:
    d2[i,j] = max(||x_i||^2 + ||x_j||^2 - 2 x_i.x_j, EPS)
    a = d2/(2k)/sigma^2 ; b = d2/(2k)/omega^2 ; c1 = k/2-1
    per_pair = same ? (-c1*log(a) + a/2) : (c1*log(b) - b/2)
    loss = sum_{i<j} per_pair

Split: everything linear in d2 has a closed form the host computes exactly
in fp64 (sum_{i<j} d2 = N*sum sq - ||sum x||^2, same per label group for
sum_same d2).  The device only computes the two log sums
    S1 = sum_{i<j} ln(d2),   S2 = sum_{same,i<j} ln(d2)
so the per-pair pipeline is a 4-pass fp8 DoubleRow matmul chain -> one Ln
activation with accum.  The -sq_j/2 column bias rides INSIDE the chain:
features 1022/1023 of each rhs slab are replaced by an fp8 hi/lo split of
-sq_j/4 pairs, and a separate lhs tensor carries constant 2.0 in those two
contraction rows (the two dropped x-features perturb each d2 by ~|2 x_i,f
x_j,f| ~ 3 of ~2050 - far inside the loss tolerance, and the host's linear
terms stay exact).

Rows are globally SORTED BY LABEL, so same-label pairs live only within a
256-row block or in the 128-wide corner between consecutive blocks.  Diag
blocks use the symmetry trick: compute the FULL [128,256] tile (diagonal
clamped to d2 == MARGIN exactly via min(t, (sq-MARGIN)/2)), then
S1_diag = (sum_full - 512*ln M)/2, S2_diag = (sum_masked - 512*ln M)/2.

Sharding: 16 row-blocks of 256; the K16 block-pair graph is oriented so
every core owns one even block (8 partners) + one odd block (7 partners)
plus both within-block triangles -> identical SPMD program on all 8 cores,
per-core variation only in input data (slab permutation).  u0/u1 panels
are interleaved per slab-quad so PE consumption (~860ns/slab) never
outruns the DMA stream (~720ns/slab).
"""

import numpy as np
import ml_dtypes

N = 4096
D = 1024
P = 128
NB = 16          # row blocks
BLK = 256        # rows per block
KC = D // P      # k chunks (8)
NCORES = 8

SIGMA = 0.2
OMEGA = 1.0
K_F = float(N)
C1 = K_F / 2.0 - 1.0                      # 2047
A_C = 1.0 / (2.0 * K_F * SIGMA * SIGMA)   # 1/327.68
B_C = 1.0 / (2.0 * K_F * OMEGA * OMEGA)   # 1/8192
LOG_A = float(np.log(A_C))
LOG_B = float(np.log(B_C))
MARGIN = 128.0   # diag clamp floor; raw diag |d2| < ~70, off-diag > ~1400
CORNER_W = 128

# cross groups: (unit g, slot_start, n_slots); unit g = 2*ls + u.
# u0/u1 pairs interleaved so each slab-quad is fully consumed in order.
NSLOT = 10       # distinct blocks resident per core
# (unit, [(slot0, nslots), ...]) - each chain needs contiguous slots,
# the group's chains just share one PSUM tile and one Ln
XGROUPS = [(0, [(1, 2), (3, 2)]), (1, [(1, 2), (3, 2)]),
           (0, [(5, 2), (7, 2)]), (1, [(5, 2), (7, 2)]),
           (2, [(2, 2), (4, 2)]), (3, [(2, 2), (4, 2)]),
           (2, [(6, 2), (9, 1)]), (3, [(6, 2), (9, 1)])]

ACC_W = 12
COL_X = list(range(8))   # LN accums for XGROUPS
COL_DL = 8               # diag full-tile ln sum (DVE reduce)
COL_DM = 9               # diag masked sum
COL_CA = 10              # corner A masked sum
COL_CB = 11              # corner B masked sum

# K8 super-node orientation: core c owns 3 super-edges (first one is
# c+1 so the consecutive-block corner lands at slot 2), plus one
# crosswise-split super-pair (perfect matching, no consecutive pairs).
# Verified to cover all 120 block pairs exactly once.
OWNED = {0: [1, 7, 6], 1: [2, 6, 7], 2: [3, 4, 5], 3: [4, 0, 5],
         4: [5, 0, 1], 5: [6, 0, 1], 6: [7, 3, 2], 7: [2, 3, 4]}
MATCH = {0: 2, 2: 0, 1: 3, 3: 1, 4: 6, 6: 4, 5: 7, 7: 5}


def _core_slabs(d):
    """Slot -> block id (10 slots): own pair, the 6 blocks of the 3
    owned super-edges, then the two crosswise partner blocks (l0's at
    slot 8, l1's at slot 9)."""
    slabs = [2 * d, 2 * d + 1]
    for o in OWNED[d]:
        slabs += [2 * o, 2 * o + 1]
    cp = MATCH[d]
    if d < cp:
        slabs += [2 * cp, 2 * cp + 1]
    else:
        slabs += [2 * cp + 1, 2 * cp]
    assert len(slabs) == NSLOT and len(set(slabs)) == NSLOT
    return slabs


_PROG_CACHE = {}


def _build_program():
    if "nc" in _PROG_CACHE:
        return _PROG_CACHE["nc"]
    import concourse.bass as bass  # noqa: F401
    import concourse.bacc as bacc
    import concourse.mybir as mybir
    import concourse.tile as tile

    F32 = mybir.dt.float32
    BF16 = mybir.dt.bfloat16
    FP8 = mybir.dt.float8e4
    AF = mybir.ActivationFunctionType
    ALU = mybir.AluOpType
    DR = mybir.MatmulPerfMode.DoubleRow

    nc = bacc.Bacc("TRN2", target_bir_lowering=False, debug=False,
                   num_devices=NCORES)
    xtp_d = nc.dram_tensor("xtp", [NSLOT, P, KC, BLK], FP8,
                           kind="ExternalInput").ap()
    lhs_d = nc.dram_tensor("lhsx", [P, 4, 2, P], FP8,
                           kind="ExternalInput").ap()
    lab_d = nc.dram_tensor("lab", [1, 640], BF16, kind="ExternalInput").ap()
    rowd_d = nc.dram_tensor("rowd", [P, 4 * 3], F32, kind="ExternalInput").ap()
    out_d = nc.dram_tensor("out", [1, ACC_W], F32, kind="ExternalOutput").ap()

    with tile.TileContext(nc) as tc:
        with (
            tc.tile_pool(name="persist", bufs=1) as persist,
            tc.tile_pool(name="ltpool", bufs=3) as ltpool,
            tc.tile_pool(name="psum", bufs=4, space="PSUM") as psum,
        ):
            # slab-major SBUF layout: per partition each slab is a
            # contiguous 2KB run -> 128x2KB DMA descriptors per slab
            xall = persist.tile([P, NSLOT, KC, BLK], FP8, tag="xall")
            lhsx = persist.tile([P, 4, 2, P], FP8, tag="lhsx")
            labb = persist.tile([P, 640], F32, tag="labb")
            labr = persist.tile([1, 640], BF16, tag="labr")
            rd = persist.tile([P, 4 * 3], F32, tag="rd")
            ones2 = persist.tile([2, P], BF16, tag="ones2")
            ones1f = persist.tile([P, 1], F32, tag="ones1f")
            acc = persist.tile([P, ACC_W], F32, tag="acc")
            outs = persist.tile([1, ACC_W], F32, tag="outs")
            t2d = persist.tile([P, 4, BLK], F32, tag="t2d")
            ltd = persist.tile([P, 4, BLK], F32, tag="ltd")
            maskd = persist.tile([P, 4, BLK], F32, tag="maskd")
            prodd = persist.tile([P, 4, BLK], F32, tag="prodd")
            maskc = persist.tile([P, 2, CORNER_W], F32, tag="maskc")
            prodc = persist.tile([P, 2, CORNER_W], F32, tag="prodc")
            warm = persist.tile([1, 1], F32, tag="warm")

            # DMA triggers: all slabs on sync (trigger issue ~0.65us each
            # stays ahead of PE consumption); scalar only the small inputs
            # so its queue is free for the Ln stream right away
            nc.scalar.dma_start(out=labr[:], in_=lab_d[:])
            nc.scalar.dma_start(out=rd[:], in_=rowd_d[:])
            # slab0 first: the diag chains' first passes read only xall,
            # so the lhsx (aug) pass data can trail slightly
            nc.sync.dma_start(out=xall[:, 0], in_=xtp_d[0])
            nc.sync.dma_start(out=lhsx[:], in_=lhs_d[:])
            for s in range(1, NSLOT):
                nc.sync.dma_start(out=xall[:, s], in_=xtp_d[s])

            wm8 = persist.tile([P, 512], FP8, tag="wm8")
            nc.gpsimd.memset(wm8[:], 1.0)
            nc.gpsimd.memset(ones2[:], 1.0)
            nc.gpsimd.memset(ones1f[:], 1.0)

            # force the Ln table load while DMAs stream
            nc.scalar.activation(warm[:], rd[0:1, 0:1], AF.Ln)

            # PE clock warmup during the DMA wait: dummy DoubleRow matmuls
            # lift the tensor engine out of its low p-state before real work
            wt = psum.tile([P, 1024], F32, tag="grp")
            wlhs = wm8[:, 0:256].rearrange("p (k m) -> p k m", k=2)
            wrhs = wm8[:].rearrange("p (k c) -> p k c", k=2)
            for i in range(10):
                nc.tensor.matmul(wt[:, 0:256], wlhs, wrhs,
                                 start=True, stop=True, perf_mode=DR)
            wsink = persist.tile([P, 1], F32, tag="wsink")
            nc.vector.tensor_copy(wsink[:], wt[:, 0:1])

            def sq_ap(g):
                return rd[:, 3 * g + 0:3 * g + 1]

            def lb_ap(g):
                return rd[:, 3 * g + 1:3 * g + 2]

            def th_ap(g):
                return rd[:, 3 * g + 2:3 * g + 3]

            # broadcast the 640-wide label row across partitions via PE;
            # two separate PSUM tiles so the matmuls don't serialize on
            # the DVE copies through a shared-tile dependency
            for lo, w in ((0, 512), (512, 128)):
                pl = psum.tile([P, 1024], F32, tag="grp")
                nc.tensor.matmul(pl[:, 0:w], ones2[0:1, :],
                                 labr[0:1, lo:lo + w], start=True, stop=True)
                nc.vector.tensor_copy(labb[:, lo:lo + w], pl[:, 0:w])
            # second warmup burst bridges until slab0's DMA semaphore so
            # the tensor engine's clock ramp never resets before the
            # real chains start
            wt2 = psum.tile([P, 1024], F32, tag="grp")
            for i in range(2):
                nc.tensor.matmul(wt2[:, 0:256], wlhs, wrhs,
                                 start=True, stop=True, perf_mode=DR)
            nc.vector.tensor_copy(wsink[:], wt2[:, 0:1])

            def mm_chain(t_ap, g, s0, ns):
                ls, u = g >> 1, g & 1
                for kp in range(KC // 2):
                    # last k-pair carries the aug rows -> special lhs;
                    # earlier passes read the slab data directly
                    if kp == KC // 2 - 1:
                        lhs = lhsx[:, g, :, :]
                    else:
                        lhs = xall[:, ls, 2 * kp:2 * kp + 2,
                                   P * u:P * (u + 1)]
                    nc.tensor.matmul(
                        t_ap, lhs,
                        xall[:, s0:s0 + ns, 2 * kp:2 * kp + 2, :]
                            .rearrange("p s k c -> p k s c"),
                        start=(kp == 0), stop=(kp == KC // 2 - 1),
                        perf_mode=DR)

            # diag group: full tiles, diagonal clamped to d2 == MARGIN
            t0 = psum.tile([P, 1024], F32, tag="grp")
            for g in range(4):
                mm_chain(t0[:, 256 * g:256 * (g + 1)], g, g >> 1, 1)
            for g in range(4):
                nc.vector.tensor_scalar(t2d[:, g, :],
                                        t0[:, 256 * g:256 * (g + 1)],
                                        th_ap(g), None, ALU.min)
                nc.scalar.activation(ltd[:, g, :], t2d[:, g, :], AF.Ln,
                                     bias=sq_ap(g), scale=-2.0)
            # same-label masks (labels vs per-partition lhs labels)
            for g in range(4):
                ls = g >> 1
                nc.vector.tensor_scalar(
                    maskd[:, g, :], labb[:, 256 * ls:256 * ls + 256],
                    lb_ap(g), None, ALU.is_equal)
            for j, (lo, g) in enumerate(((256, 1), (512, 3))):
                nc.vector.tensor_scalar(
                    maskc[:, j, :], labb[:, lo:lo + CORNER_W],
                    lb_ap(g), None, ALU.is_equal)
            nc.vector.tensor_reduce(
                acc[:, COL_DL:COL_DL + 1],
                ltd[:].rearrange("p a b -> p (a b)"),
                axis=mybir.AxisListType.X, op=ALU.add)
            nc.vector.tensor_tensor(prodd[:], maskd[:], ltd[:], ALU.mult)
            nc.vector.tensor_reduce(
                acc[:, COL_DM:COL_DM + 1],
                prodd[:].rearrange("p a b -> p (a b)"),
                axis=mybir.AxisListType.X, op=ALU.add)

            # cross groups: chains + one Ln per group
            for gi, (g, chains) in enumerate(XGROUPS):
                wtot = 256 * sum(ns for _, ns in chains)
                tg = psum.tile([P, 1024], F32, tag="grp")
                ofs = 0
                for s, ns in chains:
                    mm_chain(tg[:, ofs:ofs + 256 * ns], g, s, ns)
                    ofs += 256 * ns
                lt = ltpool.tile([P, 1024], F32, tag="lt")
                nc.scalar.activation(lt[:, 0:wtot], tg[:, 0:wtot], AF.Ln,
                                     bias=sq_ap(g), scale=-2.0,
                                     accum_out=acc[:, COL_X[gi]:
                                                   COL_X[gi] + 1])
                if gi == 1:   # corner A: lhs (l0,u1) x first 128 of slot 1
                    nc.vector.tensor_tensor(prodc[:, 0, :], maskc[:, 0, :],
                                            lt[:, 0:CORNER_W], ALU.mult)
                    nc.vector.tensor_reduce(
                        acc[:, COL_CA:COL_CA + 1], prodc[:, 0, :],
                        axis=mybir.AxisListType.X, op=ALU.add)
                if gi == 5:   # corner B: lhs (l1,u1) x first 128 of slot 9
                    nc.vector.tensor_tensor(prodc[:, 1, :], maskc[:, 1, :],
                                            lt[:, 0:CORNER_W], ALU.mult)
                    nc.vector.tensor_reduce(
                        acc[:, COL_CB:COL_CB + 1], prodc[:, 1, :],
                        axis=mybir.AxisListType.X, op=ALU.add)

            # collapse partitions to [1, ACC_W] so the out DMA is a single
            # descriptor instead of 128 tiny ones
            fin = psum.tile([P, 1024], F32, tag="grp")
            nc.tensor.matmul(fin[0:1, 0:ACC_W], ones1f[:], acc[:],
                             start=True, stop=True)
            nc.scalar.activation(outs[:], fin[0:1, 0:ACC_W], AF.Copy)
            nc.sync.dma_start(out=out_d[:], in_=outs[:])

    nc.compile()
    _PROG_CACHE["nc"] = nc
    return nc


def _host_prep(outputs, labels):
    """Sort rows by label, build per-core inputs + exact linear terms."""
    x = np.asarray(outputs, dtype=np.float32)
    lab = np.asarray(labels)
    assert x.shape == (N, D)
    perm = np.argsort(lab, kind="stable")
    xp = x[perm]
    labp = lab[perm].astype(np.float64)

    # label runs (sorted); corners require max run <= 128
    runs_end = np.empty(N, dtype=np.int64)
    i = 0
    max_run = 0
    while i < N:
        j = i
        while j < N and labp[j] == labp[i]:
            j += 1
        runs_end[i:j] = j
        max_run = max(max_run, j - i)
        i = j
    assert max_run <= CORNER_W, f"label run {max_run} exceeds corner width"

    xq = xp.astype(ml_dtypes.float8_e4m3)
    # True (unquantized) norms make d2 = sq_i + sq_j - 2*xq_i.xq_j unbiased:
    # the value-error correlation in ||xq||^2 cancels the ||e||^2 term.
    x64 = xp.astype(np.float64)
    sq = (x64 ** 2).sum(axis=1)

    # exact linear terms (fp64 closed form, true values)
    npairs = N * (N - 1) // 2
    ssum = x64.sum(axis=0)
    d2_all = N * sq.sum() - float(ssum @ ssum)
    nsame = 0
    d2_same = 0.0
    i = 0
    while i < N:
        j = int(runs_end[i])
        ng = j - i
        nsame += ng * (ng - 1) // 2
        sg = x64[i:j].sum(axis=0)
        d2_same += ng * sq[i:j].sum() - float(sg @ sg)
        i = j
    host_const = (C1 * npairs * LOG_B - (B_C / 2.0) * d2_all
                  - C1 * (LOG_A + LOG_B) * nsame
                  + ((A_C + B_C) / 2.0) * d2_same)

    # rhs aug rows: features 1022/1023 -> fp8 hi/lo of -sq/2 at weight 4.0
    # (e4m3 max is 240, so -sq/8 ~ -128 stays in range)
    r0 = (-sq / 8.0).astype(ml_dtypes.float8_e4m3)
    r1 = ((-sq / 2.0 - 4.0 * r0.astype(np.float64)) / 4.0).astype(
        ml_dtypes.float8_e4m3)
    sqq = -8.0 * (r0.astype(np.float64) + r1.astype(np.float64))
    xq[:, D - 2] = r0
    xq[:, D - 1] = r1
    # device diagonal: d2_raw = sq + sqq - 2*sum_{f<1022} xq^2 must clamp
    sq8p = (xq[:, :D - 2].astype(np.float64) ** 2).sum(axis=1)
    d2diag = sq + sqq - 2.0 * sq8p
    assert np.abs(d2diag).max() < MARGIN - 16, np.abs(d2diag).max()

    xt_q = np.ascontiguousarray(xq.T)                               # [D, N]

    in_maps = []
    for d in range(NCORES):
        slabs = _core_slabs(d)
        cols = np.concatenate(
            [np.arange(b * BLK, (b + 1) * BLK) for b in slabs])
        xtp = np.ascontiguousarray(
            xt_q[:, cols].reshape(KC, P, NSLOT, BLK).transpose(2, 1, 0, 3))
        # lhs tensor for the LAST k-pair only (chunks 6-7): quantized x
        # features, but rows 1022/1023 (chunk 7, partitions 126/127) hold
        # the aug weight 4.0
        lhsx = np.empty((P, 4, 2, P), dtype=ml_dtypes.float8_e4m3)
        for g, (slab, u) in enumerate(((0, 0), (0, 1), (1, 0), (1, 1))):
            rows = slabs[slab] * BLK + 128 * u + np.arange(P)
            blk = xq[rows, (KC - 2) * P:].reshape(P, 2, P)
            lhsx[:, g] = blk.transpose(2, 1, 0)    # [part, chunk, row m]
        lhsx[126, :, 1, :] = 4.0
        lhsx[127, :, 1, :] = 4.0
        # label row for slot0(256) | slot1(256) | slot2 first 128
        lcols = np.concatenate([cols[0:512], cols[2 * BLK:2 * BLK + 128]])
        labrow = labp[lcols].astype(ml_dtypes.bfloat16)[None, :]   # [1, 640]

        rowd = np.zeros((P, 4 * 3), dtype=np.float64)
        for g, (slab, u) in enumerate(((0, 0), (0, 1), (1, 0), (1, 1))):
            rows = slabs[slab] * BLK + 128 * u + np.arange(P)
            sqr = sq[rows]
            rowd[:, 3 * g + 0] = sqr
            rowd[:, 3 * g + 1] = labp[rows]
            rowd[:, 3 * g + 2] = (sqr - MARGIN) / 2.0
        in_maps.append({
            "xtp": xtp,
            "lhsx": np.ascontiguousarray(lhsx),
            "lab": np.ascontiguousarray(labrow),
            "rowd": rowd.astype(np.float32),
        })
    return in_maps, host_const


def _finalize(host_const, outs_list):
    """Combine per-core raw sums [ACC_W,1] with the host closed form."""
    lnm = float(np.log(MARGIN))
    total = np.float64(host_const)
    for o in outs_list:
        o = np.asarray(o, dtype=np.float64).reshape(-1)
        s1 = o[COL_X].sum() + (o[COL_DL] - 512.0 * lnm) / 2.0
        s2 = (o[COL_DM] - 512.0 * lnm) / 2.0 + o[COL_CA] + o[COL_CB]
        total += C1 * s1 - 2.0 * C1 * s2
    return np.asarray(total, dtype=np.float32)


def kernel(**inputs):
    from concourse.bass_utils import run_bass_kernel_spmd
    nc = _build_program()
    in_maps, host_const = _host_prep(inputs["outputs"], inputs["labels"])
    res = run_bass_kernel_spmd(nc, in_maps, core_ids=list(range(NCORES)))
    return _finalize(host_const, [r["out"] for r in res.results])
